# revision 24
# baseline (speedup 1.0000x reference)
"""Trainium2 Bass kernel for nn_DenoisingGNN (GCN message passing + all-pairs
edge logits), SPMD over 8 NeuronCores.

Strategy:
- GCN aggregation as dense count-matrix matmul: per core a [4096, 512] slice
  S_T[s, d_local] of (A + I) counts (duplicate edges deduped on host into
  (cell, count) pairs) is scattered on-device into a zeros DRAM tensor and
  used as the rhs of PE matmuls; D^-1/2 normalization applied as row scalings.
- Layer 1 -> 2 crossing and edge score vectors via AllGather on shared DRAM.
- Edge logits (triu outer sum s_l[row] + s_r[col] + b) built without per-pair
  gathers: per 128-row block, multi-offset indirect DMA gathers shifted
  windows of s_r, adds the per-row constant, and scatters chunks into the
  flat output with OOB-masked padding. Triangle rows are assigned to cores in
  length-complementary block pairs for exact load balance.
"""
import sys
from contextlib import ExitStack

import numpy as np

sys.path.insert(0, "/opt/trn_rl_repo")

import concourse.bass as bass
import concourse.mybir as mybir
import concourse.tile as tile
from concourse import bacc
from concourse.bass import IndirectOffsetOnAxis
from concourse.masks import make_identity

F32 = mybir.dt.float32
F32R = mybir.dt.float32r
BF16 = mybir.dt.bfloat16
I32 = mybir.dt.int32
AF = mybir.ActivationFunctionType
ALU = mybir.AluOpType

N, M, G, H, ND, E = 4096, 8192, 8, 128, 16, 131072
NCORES = 8
DL = N // NCORES            # dst nodes per core (512)
MAX_PERIOD = 10000.0
KS = 176                    # padded per-core scatter columns (KS*128 cells max)
MMDT = F32R                 # matmul compute dtype for GCN path: F32 / F32R / BF16
HUGE = 1 << 28

# triangle geometry (input independent)
ROWLEN = (N - 1 - np.arange(N)).clip(min=0)          # row i has N-1-i cols
ROWOFF = np.concatenate([[0], np.cumsum(ROWLEN)])    # flat start of each row
P_TOTAL = int(ROWOFF[-1])
NBLK = N // 128                                      # 32 row blocks
# core c owns blocks {c, 31-c, 15-c, 16+c} (complementary pairs -> equal size)
OWNED = [sorted({c, NBLK - 1 - c, 15 - c, 16 + c}) for c in range(NCORES)]
PLOC = P_TOTAL // NCORES
# edge op skeleton: (owned-block-slot, C, k); slots refer to sorted OWNED list
# 3 tier-A ops on the three blocks with L >= 1024, then tail tiers on the
# remaining block.  kcols = 3*4 + 8 + 8 + 16 = 44 offset columns.
TIER_CS = [2048, 2048, 2048, 2048, 1024, 1024] + [128] * 8 + [16] * 8
NOPS = len(TIER_CS)
SPAN_B = [int(ROWOFF[128 * (b + 1)] - ROWOFF[128 * b]) for b in range(NBLK)]
PADMAX = max(SPAN_B) + 2048          # uniform per-slot output size + dump zone
DUMP = PADMAX - 2048


def _edge_tables(c):
    """Per-core per-op offset tables. Uniform op schedule across cores:
    ops are (C, owned-slot) pairs; each op has one gather offset and one
    scatter offset per partition (row of the op's block)."""
    blocks = OWNED[c]
    tailb = NBLK - 1 - c
    bigb = sorted(b for b in blocks if b != tailb)
    # (C, block, chunk j, end_aligned?) schedule - same shape for all cores
    sched = []
    b0 = [b for b in bigb if b <= 7 or 24 > b >= 8][0]  # block c or 15-c group
    # big blocks: two C=2048 blocks (c, 15-c), one C=1024 block (16+c)
    big2048 = sorted(b for b in bigb if b <= 15)
    big1024 = [b for b in bigb if b >= 16]
    assert len(big2048) == 2 and len(big1024) == 1, (c, blocks)
    for b in big2048:
        sched += [(2048, b, 0), (2048, b, 1)]
    for b in big1024:
        sched += [(1024, b, 0), (1024, b, 1)]
    for j in range(8):
        sched.append((128, tailb, j))
    for j in range(8):
        sched.append((16, NBLK - 1, j))      # block 31; masked unless owned
    NOPS_ = len(sched)
    assert NOPS_ == NOPS
    woff = np.zeros((128, NOPS), np.int32)
    doff = np.full((128, NOPS), DUMP, np.int32)
    slot_of_op = []
    lbase = {}
    acc = 0
    for b in blocks:
        lbase[b] = acc
        acc += int(ROWOFF[128 * (b + 1)] - ROWOFF[128 * b])
    assert acc == PLOC
    for i, (Cc, b, j) in enumerate(sched):
        slot_of_op.append(blocks.index(b) if b in blocks else 3)
        if b not in blocks:
            continue
        r0 = 128 * b
        for p in range(128):
            r = r0 + p
            L = int(ROWLEN[r])
            if L < Cc:
                continue
            nch = -(-L // Cc)
            if nch > (8 if Cc <= 128 else 2) or j >= nch:
                continue
            base = int(ROWOFF[r] - ROWOFF[r0])
            if j == nch - 1:
                woff[p, i] = r + 1 + L - Cc
                doff[p, i] = base + L - Cc
            else:
                woff[p, i] = r + 1 + Cc * j
                doff[p, i] = base + Cc * j
    # cell op: all cells of rows with L < 16 (block 31 only)
    cellw = np.zeros((128, 1), np.int32)
    celld = np.full((128, 1), DUMP, np.int32)
    cellc = np.zeros((128, 1), np.int32)
    if NBLK - 1 in blocks:
        k = 0
        for r in range(N - 16, N):
            L = int(ROWLEN[r])
            base = int(ROWOFF[r] - ROWOFF[128 * (NBLK - 1)])
            for j in range(L):
                assert k < 128
                cellw[k, 0] = r + 1 + j
                celld[k, 0] = base + j
                cellc[k, 0] = r
                k += 1
    # coverage check per owned-block slot: tiers + cell + fix cover each span
    cov = [np.zeros(PADMAX + max(TIER_CS) + 8, np.int64) for _ in range(4)]
    for i, (Cc, b, j) in enumerate(sched):
        sl = slot_of_op[i]
        for p in range(128):
            if doff[p, i] < DUMP:
                cov[sl][doff[p, i]:doff[p, i] + Cc] += 1
    for k in range(128):
        if celld[k, 0] < DUMP:
            cov[3][celld[k, 0]] += 1
    # fix op covers the last cell of every row; slot j writes block[j] tensor
    srcrow = _triu_src_rows()
    fixw = np.zeros((128, 4), np.int32)
    fixd = np.full((128, 4), DUMP, np.int32)
    for jj, b in enumerate(blocks):
        for p in range(128):
            r = 128 * b + p
            if r < N - 1:
                fixw[p, jj] = srcrow[r]
                fixd[p, jj] = int(ROWOFF[r] - ROWOFF[128 * b]) \
                    + int(ROWLEN[r]) - 1
                cov[jj][fixd[p, jj]] += 1
    for jj, b in enumerate(blocks):
        assert (cov[jj][:SPAN_B[b]] >= 1).all(), f"core {c} slot {jj} uncovered"
    return woff, doff, slot_of_op, blocks, lbase, fixw, fixd, cellw, celld, cellc


def _build(slot_maps):
    """Build the SPMD bass program. slot_maps: per-op owned-slot index lists
    (identical across cores by construction)."""
    nc = bacc.Bacc()
    di = lambda n, s: nc.dram_tensor(n, s, I32, kind="ExternalInput")
    df = lambda n, s: nc.dram_tensor(n, s, F32, kind="ExternalInput")
    # values
    x_d = df("x", [N, ND])
    sfull_d = nc.dram_tensor("sfull", [N, DL], BF16, kind="ExternalInput")
    degT_d = df("degT", [128, NBLK])        # deg[128b+p] global
    degL_d = df("degL", [1, DL])            # deg of this core's dst slice
    bm_d = df("bm", [1, N])                 # batch_map as f32
    tcol_d = df("tcol", [G, 1])             # timestep as f32
    freqs_d = df("freqs", [G, 64])          # timestep freqs (replicated rows)
    iota8_d = df("iota8", [G, 1])           # 0..7 column
    iota8r_d = df("iota8r", [128, 512])     # tile(0..7, 64) per partition
    cx_d = df("cx", [M, ND + 2])            # [obj_x | obj_pos | ones | 0pad]
    nuc_d = df("nuc", [128, M // 128])
    cen_d = df("cen", [128, M // 128])
    bkb_d = df("bkb", [128, M // 128])
    obt_d = df("obt", [128, M // 128])
    wnode_d = df("Wnode", [ND, H])
    wcond_d = df("Wcond", [H, H])
    wtime_d = df("Wtime", [H, H])
    wconv1_d = df("Wconv1", [H, H])
    wconv2_d = df("Wconv2", [H, H])
    wout_d = df("Wout", [H, ND])
    bnode_d = df("bnode", [1, H])
    bcond_d = df("bcond", [1, H])
    btime_d = df("btime", [1, H])
    bconv1_d = df("bconv1", [128, 1])
    bconv2_d = df("bconv2", [128, 1])
    bout_d = df("bout", [1, ND])
    bedgev_d = df("bedgev", [1, 2])         # [b_edge, 0]
    wpair_d = df("wpair", [H, 2])           # [w_edge[:H] | w_edge[H:]]
    ewoff_d = di("ewoff", [128, NOPS])      # edge gather offsets (per op)
    edoff_d = di("edoff", [128, NOPS])      # edge scatter offsets (per op)
    cellw_d = di("cellw", [128, 1])
    celld_d = di("celld", [128, 1])
    cellc_d = di("cellc", [128, 1])
    slwoff_d = di("slwoff", [128, 4])       # s_l column gather offsets
    fixw_d = di("fixw", [128, 4])           # last-cell fix: s_l source rows
    fixd_d = di("fixd", [128, 4])           # last-cell fix: local dests
    sr1off_d = di("sr1off", [128, 1])       # all N-1 (gather s_r[N-1] col)
    # outputs
    nout_d = nc.dram_tensor("nout", [DL, ND], F32, kind="ExternalOutput")
    elog_ts = [nc.dram_tensor(f"elog{j}", [PADMAX, 1], F32,
                              kind="ExternalOutput") for j in range(4)]
    # internal
    h1loc_d = nc.dram_tensor("h1loc", [H, DL], F32)
    h1all_d = nc.dram_tensor("h1all", [NCORES, H, DL], F32, addr_space="Shared")
    slrloc_d = nc.dram_tensor("slrloc", [2, DL], F32)
    slrall_d = nc.dram_tensor("slrall", [NCORES, 2, DL], F32, addr_space="Shared")
    sl_d = nc.dram_tensor("sl_d", [N, 1], F32)
    sr_d = nc.dram_tensor("sr_d", [N, 1], F32)

    MT = 64  # M // 128 tiles

    with tile.TileContext(nc) as tc:
        with (
            tc.tile_pool(name="cst", bufs=1) as cst,
            tc.tile_pool(name="wts", bufs=1) as wts,
            tc.tile_pool(name="ssb", bufs=1) as ssb,
            tc.tile_pool(name="ps", bufs=2, space="PSUM") as ps,
            tc.tile_pool(name="pst", bufs=2, space="PSUM") as pst,
        ):
            ident = cst.tile([128, 128], F32)
            make_identity(nc, ident[:])
            ident_m = cst.tile([128, 128], MMDT)
            nc.vector.tensor_copy(out=ident_m[:], in_=ident[:])
            ones = cst.tile([128, 520], F32)
            nc.gpsimd.memset(ones[:], 1.0)
            ones_m = cst.tile([128, 8], MMDT)
            nc.vector.tensor_copy(out=ones_m[:], in_=ones[:, 0:8])
            pit = cst.tile([128, 1], F32)
            nc.gpsimd.memset(pit[:], float(np.pi))

            # ---------------- load S_T into SBUF [128, 32, 512] ----------------
            s_sb = ssb.tile([128, NBLK, DL], MMDT)
            nc.gpsimd.dma_start(
                out=s_sb[:], in_=sfull_d[:].rearrange("(t p) d -> p t d", p=128))

            # ---------------- weights / misc loads ----------------
            ld = (nc.gpsimd.dma_start if MMDT != F32 else nc.sync.dma_start)
            wnode = wts.tile([ND, H], MMDT)
            ld(out=wnode[:], in_=wnode_d[:])
            wcond = wts.tile([H, H], MMDT)
            ld(out=wcond[:], in_=wcond_d[:])
            wtime = wts.tile([H, H], MMDT)
            ld(out=wtime[:], in_=wtime_d[:])
            wconv1 = wts.tile([H, H], MMDT)
            ld(out=wconv1[:], in_=wconv1_d[:])
            wconv2 = wts.tile([H, H], MMDT)
            ld(out=wconv2[:], in_=wconv2_d[:])
            wout = wts.tile([H, ND], F32)
            nc.sync.dma_start(out=wout[:], in_=wout_d[:])
            wpair = wts.tile([H, 2], F32)
            nc.sync.dma_start(out=wpair[:], in_=wpair_d[:])
            bnode = wts.tile([1, H], MMDT)
            ld(out=bnode[:], in_=bnode_d[:])
            bcond = wts.tile([1, H], MMDT)
            ld(out=bcond[:], in_=bcond_d[:])
            btime = wts.tile([1, H], MMDT)
            ld(out=btime[:], in_=btime_d[:])
            bconv1 = wts.tile([128, 1], F32)
            nc.sync.dma_start(out=bconv1[:], in_=bconv1_d[:])
            bconv2 = wts.tile([128, 1], F32)
            nc.sync.dma_start(out=bconv2[:], in_=bconv2_d[:])
            bout = wts.tile([1, ND], F32)
            nc.sync.dma_start(out=bout[:], in_=bout_d[:])
            bedgev = wts.tile([1, 2], F32)
            nc.sync.dma_start(out=bedgev[:], in_=bedgev_d[:])

            # ---------------- x transpose ----------------
            early_ctx = ExitStack()
            early = early_ctx.enter_context(tc.tile_pool(name="early", bufs=1))
            xt_sb = early.tile([ND, N], MMDT)
            xtiles = early.tile([128, NBLK, ND], F32)
            nc.sync.dma_start(
                out=xtiles[:], in_=x_d[:].rearrange("(t p) nd -> p t nd", p=128))
            for q in range(8):
                xtp = pst.tile([ND, 512], F32, space="PSUM", tag="tp")
                for u in range(4):
                    t = 4 * q + u
                    nc.tensor.transpose(
                        out=xtp[:, 128 * u:128 * (u + 1)],
                        in_=xtiles[:, t, :], identity=ident[:])
                nc.vector.tensor_copy(out=xt_sb[:, 512 * q:512 * (q + 1)], in_=xtp[:])

            # ---------------- dinv (global, col layout) ----------------
            degT = wts.tile([128, NBLK], F32)
            nc.sync.dma_start(out=degT[:], in_=degT_d[:])
            dinvT = wts.tile([128, NBLK], F32)
            nc.scalar.activation(dinvT[:], degT[:], AF.Sqrt)
            nc.vector.reciprocal(out=dinvT[:], in_=dinvT[:])
            # dinv for local dst slice, replicated [128, DL]
            degL = wts.tile([1, DL], F32)
            nc.sync.dma_start(out=degL[:], in_=degL_d[:])
            dinvLp = ps.tile([128, DL], F32, space="PSUM", tag="big")
            for u in range(4):
                nc.tensor.matmul(dinvLp[:, 128 * u:128 * (u + 1)],
                                 ones[0:1, 0:128], degL[:, 128 * u:128 * (u + 1)],
                                 start=True, stop=True)
            dinvL = wts.tile([128, DL], F32)
            nc.scalar.activation(dinvL[:], dinvLp[:], AF.Sqrt)
            nc.vector.reciprocal(out=dinvL[:], in_=dinvL[:])

            # ---------------- selmap_T [8, N] ----------------
            bmrow = wts.tile([1, N], F32)
            nc.sync.dma_start(out=bmrow[:], in_=bm_d[:])
            iota8 = wts.tile([G, 1], F32)
            nc.sync.dma_start(out=iota8[:], in_=iota8_d[:])
            selmap = early.tile([G, N], MMDT)
            for q in range(8):
                bmp = pst.tile([G, 512], F32, space="PSUM", tag="tp")
                nc.tensor.matmul(bmp[:], ones[0:1, 0:G],
                                 bmrow[:, 512 * q:512 * (q + 1)],
                                 start=True, stop=True)
                nc.vector.tensor_tensor(
                    out=selmap[:, 512 * q:512 * (q + 1)], in0=bmp[:],
                    in1=iota8[:].to_broadcast([G, 512]), op=ALU.is_equal)

            # ---------------- time embedding ----------------
            tcol = wts.tile([G, 1], F32)
            nc.sync.dma_start(out=tcol[:], in_=tcol_d[:])
            freqs = wts.tile([G, 64], F32)
            nc.sync.dma_start(out=freqs[:], in_=freqs_d[:])
            targ = wts.tile([G, 64], F32)
            nc.vector.tensor_tensor(out=targ[:], in0=freqs[:],
                                    in1=tcol[:].to_broadcast([G, 64]), op=ALU.mult)

            P2 = float(2 * np.pi)

            def reduce_to_pi(xs):
                # range-reduce xs to (-pi, pi] (robust to cast rounding mode)
                y = wts.tile([G, 64], F32, tag="rr_y")
                nc.vector.tensor_scalar(out=y[:], in0=xs[:], scalar1=1.0 / P2,
                                        scalar2=None, op0=ALU.mult)
                yi = wts.tile([G, 64], I32, tag="rr_yi")
                nc.vector.tensor_copy(out=yi[:], in_=y[:])
                yf = wts.tile([G, 64], F32, tag="rr_yf")
                nc.vector.tensor_copy(out=yf[:], in_=yi[:])
                nc.vector.tensor_scalar(out=yf[:], in0=yf[:], scalar1=-P2,
                                        scalar2=None, op0=ALU.mult)
                nc.vector.tensor_tensor(out=xs[:], in0=xs[:], in1=yf[:],
                                        op=ALU.add)
                adj = wts.tile([G, 64], F32, tag="rr_adj")
                nc.vector.tensor_scalar(out=adj[:], in0=xs[:],
                                        scalar1=float(np.pi), scalar2=None,
                                        op0=ALU.is_gt)
                nc.vector.tensor_scalar(out=adj[:], in0=adj[:], scalar1=-P2,
                                        scalar2=None, op0=ALU.mult)
                nc.vector.tensor_tensor(out=xs[:], in0=xs[:], in1=adj[:],
                                        op=ALU.add)
                nc.vector.tensor_scalar(out=adj[:], in0=xs[:],
                                        scalar1=float(-np.pi), scalar2=None,
                                        op0=ALU.is_lt)
                nc.vector.tensor_scalar(out=adj[:], in0=adj[:], scalar1=P2,
                                        scalar2=None, op0=ALU.mult)
                nc.vector.tensor_tensor(out=xs[:], in0=xs[:], in1=adj[:],
                                        op=ALU.add)

            temb = wts.tile([G, H], F32)
            # cos(x) = sin(x + pi/2) -> [:, :64]
            cosm = wts.tile([G, 64], F32)
            nc.vector.tensor_scalar(out=cosm[:], in0=targ[:],
                                    scalar1=float(np.pi / 2), scalar2=None,
                                    op0=ALU.add)
            reduce_to_pi(cosm)
            nc.scalar.activation(temb[:, 0:64], cosm[:], AF.Sin)
            # sin -> [:, 64:]
            sinm = wts.tile([G, 64], F32)
            nc.vector.tensor_copy(out=sinm[:], in_=targ[:])
            reduce_to_pi(sinm)
            nc.scalar.activation(temb[:, 64:128], sinm[:], AF.Sin)
            tembT_p = pst.tile([H, G], F32, space="PSUM", tag="tp")
            nc.tensor.transpose(out=tembT_p[:], in_=temb[:], identity=ident[0:G, 0:G])
            tembT = wts.tile([H, G], MMDT)
            nc.vector.tensor_copy(out=tembT[:], in_=tembT_p[:])

            # ---------------- cond pooling ----------------
            cx = early.tile([128, MT, ND + 2], MMDT)
            (nc.gpsimd.dma_start if MMDT != F32 else nc.sync.dma_start)(
                out=cx[:], in_=cx_d[:].rearrange("(t p) d -> p t d", p=128))
            nuc = wts.tile([128, MT], F32)
            nc.sync.dma_start(out=nuc[:], in_=nuc_d[:])
            cen = wts.tile([128, MT], F32)
            nc.sync.dma_start(out=cen[:], in_=cen_d[:])
            bkb = wts.tile([128, MT], F32)
            nc.sync.dma_start(out=bkb[:], in_=bkb_d[:])
            obt = wts.tile([128, MT], F32)
            nc.sync.dma_start(out=obt[:], in_=obt_d[:])
            iota8r = wts.tile([128, 512], F32)
            nc.sync.dma_start(out=iota8r[:], in_=iota8r_d[:])
            isl = wts.tile([128, MT], F32)
            nc.vector.tensor_scalar(out=isl[:], in0=nuc[:], scalar1=0.0,
                                    scalar2=None, op0=ALU.is_equal)
            isr = wts.tile([128, MT], F32)
            nc.vector.tensor_scalar(out=isr[:], in0=nuc[:], scalar1=2.0,
                                    scalar2=None, op0=ALU.is_equal)
            isb = wts.tile([128, MT], F32)
            nc.vector.tensor_scalar(out=isb[:], in0=bkb[:], scalar1=0.0,
                                    scalar2=None, op0=ALU.is_equal)
            t1 = wts.tile([128, MT], F32)
            nc.vector.tensor_tensor(out=t1[:], in0=cen[:], in1=isb[:], op=ALU.mult)
            t2 = wts.tile([128, MT], F32)
            nc.vector.tensor_tensor(out=t2[:], in0=isr[:], in1=isb[:], op=ALU.mult)
            cm = wts.tile([128, MT], F32)
            nc.vector.tensor_tensor(out=cm[:], in0=isl[:], in1=t1[:], op=ALU.max)
            nc.vector.tensor_tensor(out=cm[:], in0=cm[:], in1=t2[:], op=ALU.max)
            sel = early.tile([128, MT, G], F32)
            nc.vector.tensor_tensor(
                out=sel[:], in0=obt[:, :, None].to_broadcast([128, MT, G]),
                in1=iota8r[:].rearrange("p (t g) -> p t g", g=G), op=ALU.is_equal)
            nc.vector.tensor_tensor(
                out=sel[:], in0=sel[:],
                in1=cm[:, :, None].to_broadcast([128, MT, G]), op=ALU.mult)
            sel_m = early.tile([128, MT, G], MMDT)
            nc.vector.tensor_copy(out=sel_m[:], in_=sel[:])
            poolp = ps.tile([G, ND + 2], F32, space="PSUM", tag="acc8")
            for t in range(MT):
                nc.tensor.matmul(poolp[:], sel_m[:, t, :], cx[:, t, :],
                                 start=(t == 0), stop=(t == MT - 1))
            pooled = wts.tile([G, ND + 2], F32)
            nc.vector.tensor_copy(out=pooled[:], in_=poolp[:])
            cnt = wts.tile([G, 1], F32)
            nc.vector.tensor_scalar(out=cnt[:], in0=pooled[:, ND:ND + 1],
                                    scalar1=1.0, scalar2=None, op0=ALU.max)
            rcnt = wts.tile([G, 1], F32)
            nc.vector.reciprocal(out=rcnt[:], in_=cnt[:])
            pxT_p = pst.tile([ND, G], F32, space="PSUM", tag="tp")
            nc.tensor.transpose(out=pxT_p[:], in_=pooled[:, 0:ND],
                                identity=ident[0:G, 0:G])
            pxT = wts.tile([ND, G], MMDT)
            nc.vector.tensor_copy(out=pxT[:], in_=pxT_p[:])
            cT_p = pst.tile([1, G], F32, space="PSUM", tag="tp")
            nc.tensor.transpose(out=cT_p[:], in_=pooled[:, ND:ND + 1],
                                identity=ident[0:G, 0:G])
            cT = wts.tile([1, G], MMDT)
            nc.vector.tensor_copy(out=cT[:], in_=cT_p[:])
            p1 = ps.tile([G, H], F32, space="PSUM", tag="acc8")
            nc.tensor.matmul(p1[:], pxT[:], wnode[:], start=True, stop=False)
            nc.tensor.matmul(p1[:], cT[:], bnode[:], start=False, stop=True)
            pmean = wts.tile([G, H], F32)
            nc.vector.tensor_tensor(out=pmean[:], in0=p1[:],
                                    in1=rcnt[:].to_broadcast([G, H]), op=ALU.mult)
            pmT_p = pst.tile([H, G], F32, space="PSUM", tag="tp")
            nc.tensor.transpose(out=pmT_p[:], in_=pmean[:], identity=ident[0:G, 0:G])
            pmT = wts.tile([H, G], MMDT)
            nc.vector.tensor_copy(out=pmT[:], in_=pmT_p[:])

            # ---------------- combined [8, H] ----------------
            combp = ps.tile([G, H], F32, space="PSUM", tag="acc8")
            nc.tensor.matmul(combp[:], tembT[:], wtime[:], start=True, stop=False)
            nc.tensor.matmul(combp[:], pmT[:], wcond[:], start=False, stop=False)
            nc.tensor.matmul(combp[:], ones_m[0:1, 0:G], btime[:],
                             start=False, stop=False)
            nc.tensor.matmul(combp[:], ones_m[0:1, 0:G], bcond[:],
                             start=False, stop=False)
            nc.tensor.matmul(combp[:], ones_m[0:1, 0:G], bnode[:],
                             start=False, stop=True)
            comb = wts.tile([G, H], MMDT)
            nc.vector.tensor_copy(out=comb[:], in_=combp[:])

            # ---------------- node_emb_T then layers ----------------
            neT = ssb.tile([H, N], MMDT)
            for q in range(8):
                nep = ps.tile([H, 512], F32, space="PSUM", tag="big")
                nc.tensor.matmul(nep[:], wnode[:],
                                 xt_sb[:, 512 * q:512 * (q + 1)],
                                 start=True, stop=False)
                nc.tensor.matmul(nep[:], comb[:],
                                 selmap[:, 512 * q:512 * (q + 1)],
                                 start=False, stop=True)
                nc.vector.tensor_copy(out=neT[:, 512 * q:512 * (q + 1)], in_=nep[:])
            early_ctx.close()

            h1 = ssb.tile([H, DL], F32)
            h2 = ssb.tile([H, DL], F32)
            h1T = ssb.tile([H, N], MMDT)
            ztiles = ssb.tile([128, NBLK, H], MMDT, tag="ztiles")

            for layer in range(2):
                wconv = wconv1 if layer == 0 else wconv2
                bconv = bconv1 if layer == 0 else bconv2
                src = neT if layer == 0 else h1T
                hout = h1 if layer == 0 else h2
                # proj_T = Wconv^T @ src, written back in place chunk by chunk
                for q in range(8):
                    prp = ps.tile([H, 512], F32, space="PSUM", tag="big")
                    nc.tensor.matmul(prp[:], wconv[:],
                                     src[:, 512 * q:512 * (q + 1)],
                                     start=True, stop=True)
                    nc.vector.tensor_copy(out=src[:, 512 * q:512 * (q + 1)],
                                          in_=prp[:])
                # z tiles = transpose(proj_T) * dinv col
                for t in range(NBLK):
                    ztp = pst.tile([128, 128], MMDT, space="PSUM", tag="ztp")
                    nc.tensor.transpose(out=ztp[:],
                                        in_=src[:, 128 * t:128 * (t + 1)],
                                        identity=ident_m[:])
                    nc.vector.tensor_tensor(
                        out=ztiles[:, t, :], in0=ztp[:],
                        in1=dinvT[:, t:t + 1].to_broadcast([128, H]), op=ALU.mult)
                # aggregate: agg_T[h, d_local] += z_t^T S_t
                aggp = ps.tile([H, DL], F32, space="PSUM", tag="big")
                for t in range(NBLK):
                    nc.tensor.matmul(aggp[:], ztiles[:, t, :], s_sb[:, t, :],
                                     start=(t == 0), stop=(t == NBLK - 1))
                # h = relu(agg * dinvL + b)
                nc.vector.tensor_tensor(out=hout[:], in0=aggp[:], in1=dinvL[:],
                                        op=ALU.mult)
                nc.scalar.activation(hout[:], hout[:], AF.Relu, bias=bconv[:],
                                     scale=1.0)
                if layer == 0:
                    nc.sync.dma_start(out=h1loc_d[:], in_=hout[:])
                    nc.gpsimd.collective_compute(
                        "AllGather", ALU.bypass,
                        replica_groups=[list(range(NCORES))],
                        ins=[h1loc_d[:]], outs=[h1all_d[:]])
                    for q in range(NCORES):
                        (nc.gpsimd.dma_start if MMDT != F32
                         else nc.sync.dma_start)(
                            out=h1T[:, DL * q:DL * (q + 1)], in_=h1all_d[q])

            # ---------------- outputs: node noise ----------------
            no_sb = wts.tile([128, 4, ND], F32)
            for u in range(4):
                nop = pst.tile([128, ND], F32, space="PSUM", tag="ztp")
                nc.tensor.matmul(nop[:], h2[:, 128 * u:128 * (u + 1)],
                                 wout[:], start=True, stop=False)
                nc.tensor.matmul(nop[:], ones[0:1, 0:128], bout[:],
                                 start=False, stop=True)
                nc.vector.tensor_copy(out=no_sb[:, u, :], in_=nop[:])
            nc.sync.dma_start(
                out=nout_d[:].rearrange("(u p) d -> p u d", p=128), in_=no_sb[:])

            # ---------------- s_l / s_r + allgather ----------------
            slrp = ps.tile([2, DL], F32, space="PSUM", tag="big")
            nc.tensor.matmul(slrp[:], wpair[:], h2[:],
                             start=True, stop=False)
            nc.tensor.matmul(slrp[:], bedgev[:], ones[0:1, 0:DL],
                             start=False, stop=True)
            slr = wts.tile([2, DL], F32)
            nc.vector.tensor_copy(out=slr[:], in_=slrp[:])
            nc.sync.dma_start(out=slrloc_d[:], in_=slr[:])
            nc.gpsimd.collective_compute(
                "AllGather", ALU.bypass, replica_groups=[list(range(NCORES))],
                ins=[slrloc_d[:]], outs=[slrall_d[:]])
            nc.sync.dma_start(
                out=sl_d[:, 0].rearrange("(a b) -> a b", a=NCORES),
                in_=slrall_d[:, 0, :])
            nc.sync.dma_start(
                out=sr_d[:, 0].rearrange("(a b) -> a b", a=NCORES),
                in_=slrall_d[:, 1, :])

        # ---------------- edge logits ----------------
        with (
            tc.tile_pool(name="esb", bufs=3) as esb,
            tc.tile_pool(name="ecst", bufs=1) as ecst,
        ):
            ewoff = ecst.tile([128, NOPS], I32)
            nc.sync.dma_start(out=ewoff[:], in_=ewoff_d[:])
            edoff = ecst.tile([128, NOPS], I32)
            nc.sync.dma_start(out=edoff[:], in_=edoff_d[:])
            slwoff = ecst.tile([128, 4], I32)
            nc.sync.dma_start(out=slwoff[:], in_=slwoff_d[:])
            slcols = ecst.tile([128, 4], F32)
            for j in range(4):
                nc.gpsimd.indirect_dma_start(
                    out=slcols[:, j:j + 1], out_offset=None, in_=sl_d[:],
                    in_offset=IndirectOffsetOnAxis(ap=slwoff[:, j:j + 1],
                                                   axis=0))
            for i, Cc in enumerate(TIER_CS):
                slot = slot_maps[i]
                pay = esb.tile([128, 2048], F32, tag="pay")
                nc.gpsimd.indirect_dma_start(
                    out=pay[:, 0:Cc], out_offset=None, in_=sr_d[:],
                    in_offset=IndirectOffsetOnAxis(ap=ewoff[:, i:i + 1],
                                                   axis=0))
                nc.vector.tensor_tensor(
                    out=pay[:, 0:Cc], in0=pay[:, 0:Cc],
                    in1=slcols[:, slot:slot + 1].to_broadcast([128, Cc]),
                    op=ALU.add)
                nc.gpsimd.indirect_dma_start(
                    out=elog_ts[slot][:],
                    out_offset=IndirectOffsetOnAxis(ap=edoff[:, i:i + 1],
                                                    axis=0),
                    in_=pay[:, 0:Cc], in_offset=None)
            # cell op (rows with L < 16; only the core owning block 31)
            cellw = ecst.tile([128, 1], I32)
            nc.sync.dma_start(out=cellw[:], in_=cellw_d[:])
            celld = ecst.tile([128, 1], I32)
            nc.sync.dma_start(out=celld[:], in_=celld_d[:])
            cellc = ecst.tile([128, 1], I32)
            nc.sync.dma_start(out=cellc[:], in_=cellc_d[:])
            cw = ecst.tile([128, 1], F32)
            nc.gpsimd.indirect_dma_start(
                out=cw[:], out_offset=None, in_=sr_d[:],
                in_offset=IndirectOffsetOnAxis(ap=cellw[:], axis=0))
            cc = ecst.tile([128, 1], F32)
            nc.gpsimd.indirect_dma_start(
                out=cc[:], out_offset=None, in_=sl_d[:],
                in_offset=IndirectOffsetOnAxis(ap=cellc[:], axis=0))
            cv = ecst.tile([128, 1], F32)
            nc.vector.tensor_tensor(out=cv[:], in0=cw[:], in1=cc[:], op=ALU.add)
            nc.gpsimd.indirect_dma_start(
                out=elog_ts[3][:],
                out_offset=IndirectOffsetOnAxis(ap=celld[:], axis=0),
                in_=cv[:], in_offset=None)
            # last-cell fix (ordered after tier scatters via DRAM WAW deps)
            fixw = ecst.tile([128, 4], I32)
            nc.sync.dma_start(out=fixw[:], in_=fixw_d[:])
            fixd = ecst.tile([128, 4], I32)
            nc.sync.dma_start(out=fixd[:], in_=fixd_d[:])
            sr1off = ecst.tile([128, 1], I32)
            nc.sync.dma_start(out=sr1off[:], in_=sr1off_d[:])
            g2 = ecst.tile([128, 1], F32)
            nc.gpsimd.indirect_dma_start(
                out=g2[:], out_offset=None, in_=sr_d[:],
                in_offset=IndirectOffsetOnAxis(ap=sr1off[:], axis=0))
            for j in range(4):
                g1 = ecst.tile([128, 1], F32, tag="g1")
                nc.gpsimd.indirect_dma_start(
                    out=g1[:], out_offset=None, in_=sl_d[:],
                    in_offset=IndirectOffsetOnAxis(ap=fixw[:, j:j + 1],
                                                   axis=0))
                fv = ecst.tile([128, 1], F32, tag="fv")
                nc.vector.tensor_tensor(out=fv[:], in0=g1[:], in1=g2[:],
                                        op=ALU.add)
                nc.gpsimd.indirect_dma_start(
                    out=elog_ts[j][:],
                    out_offset=IndirectOffsetOnAxis(ap=fixd[:, j:j + 1],
                                                    axis=0),
                    in_=fv[:], in_offset=None)
    return nc


_CACHE = {}
_TRIU = {}


def _triu_src_rows():
    """Per-row source row for the LAST cell of each triu row, exactly as
    jnp.triu_indices produces it (it wraps the last cell of row r to
    (r+1, -1) for some rows). Input-independent; computed once via jax."""
    if "src" in _TRIU:
        return _TRIU["src"]
    import jax
    import jax.numpy as jnp
    cpu = jax.local_devices(backend="cpu")[0]
    with jax.default_device(cpu):
        jrow, jcol = jnp.triu_indices(N, k=1)
        jrow = np.asarray(jrow).astype(np.int64)
        jcol = np.asarray(jcol).astype(np.int64)
    # verify: all non-last positions are plain row-major
    rr = np.searchsorted(ROWOFF, np.arange(jrow.size), side="right") - 1
    cc = np.arange(jrow.size) - ROWOFF[rr] + rr + 1
    lastpos = (ROWOFF[: N - 1] + ROWLEN[: N - 1] - 1).astype(np.int64)
    ismid = np.ones(jrow.size, bool)
    ismid[lastpos] = False
    assert (jrow[ismid] == rr[ismid]).all() and (jcol[ismid] == cc[ismid]).all()
    assert np.isin(jcol[lastpos] % N, [N - 1]).all()
    _TRIU["src"] = jrow[lastpos]          # s_l row index for last cell of row r
    return _TRIU["src"]


def _prep_host(inputs):
    """Split/shard inputs into 8 per-core in_maps (host: integer index prep,
    dtype casts, reshapes only)."""
    src = np.asarray(inputs["edge_index"][0], np.int64)
    dst = np.asarray(inputs["edge_index"][1], np.int64)
    loops = np.arange(N, dtype=np.int64)
    srcA = np.concatenate([src, loops])
    dstA = np.concatenate([dst, loops])
    deg = np.bincount(dstA, minlength=N).astype(np.float32)

    x = np.asarray(inputs["x"], np.float32)
    bm = np.asarray(inputs["batch_map"], np.float32)[None, :]
    tcol = np.asarray(inputs["timestep"], np.float32)[:, None]
    half = H // 2
    freqs = np.exp(-np.log(MAX_PERIOD) *
                   np.arange(half, dtype=np.float32) / half)
    freqs_rep = np.tile(freqs[None, :], (G, 1)).astype(np.float32)
    iota8 = np.arange(G, dtype=np.float32)[:, None]
    iota8r = np.tile(np.arange(G, dtype=np.float32)[None, :], (128, 64))
    cx = np.concatenate([
        np.asarray(inputs["obj_x"], np.float32),
        np.asarray(inputs["obj_pos"], np.float32),
        np.ones((M, 1), np.float32),
        np.zeros((M, 1), np.float32)], axis=1)
    colsM = lambda v: np.ascontiguousarray(
        np.asarray(v, np.float32).reshape(M // 128, 128).T)
    degT = np.ascontiguousarray(deg.reshape(NBLK, 128).T)
    w_edge = np.asarray(inputs["w_edge"], np.float32)
    wpair = np.stack([w_edge[:H], w_edge[H:]], axis=1)
    bedgev = np.array([[float(np.asarray(inputs["b_edge"])), 0.0]], np.float32)

    base = dict(
        x=x, bm=bm, tcol=tcol, freqs=freqs_rep, iota8=iota8, iota8r=iota8r,
        cx=cx,
        nuc=colsM(inputs["obj_nucleotide_mask"]),
        cen=colsM(inputs["obj_central_mask"]),
        bkb=colsM(inputs["obj_backbone_mask"]),
        obt=colsM(inputs["obj_batch"]),
        degT=degT,

        Wnode=np.asarray(inputs["W_node"], np.float32),
        Wcond=np.asarray(inputs["W_cond"], np.float32),
        Wtime=np.asarray(inputs["W_time"], np.float32),
        Wconv1=np.asarray(inputs["W_conv1"], np.float32),
        Wconv2=np.asarray(inputs["W_conv2"], np.float32),
        Wout=np.asarray(inputs["W_out"], np.float32),
        bnode=np.asarray(inputs["b_node"], np.float32)[None, :],
        bcond=np.asarray(inputs["b_cond"], np.float32)[None, :],
        btime=np.asarray(inputs["b_time"], np.float32)[None, :],
        bconv1=np.asarray(inputs["b_conv1"], np.float32)[:, None],
        bconv2=np.asarray(inputs["b_conv2"], np.float32)[:, None],
        bout=np.asarray(inputs["b_out"], np.float32)[None, :],
        bedgev=bedgev, wpair=wpair,
    )

    in_maps = []
    slot_maps = None
    for c in range(NCORES):
        m = dict(base)
        lo = DL * c
        sel = (dstA >= lo) & (dstA < lo + DL)
        import ml_dtypes
        sfull = np.zeros(N * DL, np.float32)
        np.add.at(sfull, srcA[sel] * DL + (dstA[sel] - lo), 1.0)
        m["sfull"] = sfull.reshape(N, DL).astype(ml_dtypes.bfloat16)
        m["degL"] = deg[lo:lo + DL][None, :]
        (woff, doff, slots, blocks, lbase, fixw, fixd,
         cellw, celld, cellc) = _edge_tables(c)
        m["ewoff"] = woff
        m["edoff"] = doff
        m["fixw"] = fixw
        m["fixd"] = fixd
        m["cellw"] = cellw
        m["celld"] = celld
        m["cellc"] = cellc
        m["sr1off"] = np.full((128, 1), N - 1, np.int32)
        slw = np.zeros((128, 4), np.int32)
        for j, b in enumerate(blocks):
            slw[:, j] = 128 * b + np.arange(128)
        m["slwoff"] = slw
        if slot_maps is None:
            slot_maps = slots
        else:
            assert slot_maps == slots
        in_maps.append(m)
    return in_maps, slot_maps


def _assemble(results):
    nout = np.concatenate([results[c]["nout"] for c in range(NCORES)], axis=0)
    elog = np.empty(P_TOTAL, np.float32)
    for c in range(NCORES):
        for j, b in enumerate(OWNED[c]):
            span = SPAN_B[b]
            elog[int(ROWOFF[128 * b]):int(ROWOFF[128 * (b + 1)])] = \
                results[c][f"elog{j}"].ravel()[:span]
    return nout, elog


def kernel(**inputs):
    from concourse.bass_utils import run_bass_kernel_spmd
    in_maps, slot_maps = _prep_host(inputs)
    key = "prog"
    if key not in _CACHE:
        nc = _build(slot_maps)
        nc.finalize()
        _CACHE[key] = nc
    nc = _CACHE[key]
    res = run_bass_kernel_spmd(nc, in_maps, list(range(NCORES)))
    return _assemble(res.results)


# revision 29
# speedup vs baseline: 1.0547x; 1.0547x over previous
"""Trainium2 Bass kernel for nn_DenoisingGNN (GCN message passing + all-pairs
edge logits), SPMD over 8 NeuronCores.

Strategy:
- GCN aggregation as dense count-matrix matmul: per core a [4096, 512] slice
  of the (A + I) edge-count matrix (host-built from integer indices, exact in
  bf16, cast to f32r on load) is the rhs of PE matmuls with dinv-scaled
  projection rows as lhsT; D^-1/2 normalization applied as row scalings.
- Layer 1 -> 2 crossing and edge score vectors via AllGather collectives.
- Edge logits (triu outer sum s_l[row] + s_r[col] + b) built without per-pair
  gathers: per 128-row block and chunk index, a one-offset-per-partition
  indirect DMA gathers shifted windows of s_r, a DVE add applies the per-row
  constant, and an indirect scatter places overlap-consistent chunks. Each
  owned block writes its own output tensor (avoids DRAM WAW serialization);
  masked slots land in a per-tensor dump zone. A final fix pass rewrites each
  row's last cell to match jnp.triu_indices' wrap semantics. Triangle rows
  are assigned to cores in length-complementary block pairs for exact load
  balance.
"""
import sys
from contextlib import ExitStack

import numpy as np

sys.path.insert(0, "/opt/trn_rl_repo")

import concourse.bass as bass
import concourse.mybir as mybir
import concourse.tile as tile
from concourse import bacc
from concourse.bass import IndirectOffsetOnAxis
from concourse.masks import make_identity

F32 = mybir.dt.float32
F32R = mybir.dt.float32r
BF16 = mybir.dt.bfloat16
I32 = mybir.dt.int32
AF = mybir.ActivationFunctionType
ALU = mybir.AluOpType

N, M, G, H, ND, E = 4096, 8192, 8, 128, 16, 131072
NCORES = 8
DL = N // NCORES            # dst nodes per core (512)
MAX_PERIOD = 10000.0
KS = 176                    # padded per-core scatter columns (KS*128 cells max)
MMDT = F32R                 # matmul compute dtype for GCN path: F32 / F32R / BF16
HUGE = 1 << 28

# triangle geometry (input independent)
ROWLEN = (N - 1 - np.arange(N)).clip(min=0)          # row i has N-1-i cols
ROWOFF = np.concatenate([[0], np.cumsum(ROWLEN)])    # flat start of each row
P_TOTAL = int(ROWOFF[-1])
NBLK = N // 128                                      # 32 row blocks
# core c owns blocks {c, 31-c, 15-c, 16+c} (complementary pairs -> equal size)
OWNED = [sorted({c, NBLK - 1 - c, 15 - c, 16 + c}) for c in range(NCORES)]
PLOC = P_TOTAL // NCORES
# edge op skeleton: (owned-block-slot, C, k); slots refer to sorted OWNED list
# 3 tier-A ops on the three blocks with L >= 1024, then tail tiers on the
# remaining block.  kcols = 3*4 + 8 + 8 + 16 = 44 offset columns.
TIER_CS = [2048, 2048, 2048, 2048, 1024, 1024] + [128] * 8 + [16] * 8
NOPS = len(TIER_CS)
SPAN_B = [int(ROWOFF[128 * (b + 1)] - ROWOFF[128 * b]) for b in range(NBLK)]
PADMAX = max(SPAN_B) + 2048          # uniform per-slot output size + dump zone
DUMP = PADMAX - 2048


def _edge_tables(c):
    """Per-core per-op offset tables. Uniform op schedule across cores:
    ops are (C, owned-slot) pairs; each op has one gather offset and one
    scatter offset per partition (row of the op's block)."""
    blocks = OWNED[c]
    tailb = NBLK - 1 - c
    bigb = sorted(b for b in blocks if b != tailb)
    # (C, block, chunk j, end_aligned?) schedule - same shape for all cores
    sched = []
    b0 = [b for b in bigb if b <= 7 or 24 > b >= 8][0]  # block c or 15-c group
    # big blocks: two C=2048 blocks (c, 15-c), one C=1024 block (16+c)
    big2048 = sorted(b for b in bigb if b <= 15)
    big1024 = [b for b in bigb if b >= 16]
    assert len(big2048) == 2 and len(big1024) == 1, (c, blocks)
    for b in big2048:
        sched += [(2048, b, 0), (2048, b, 1)]
    for b in big1024:
        sched += [(1024, b, 0), (1024, b, 1)]
    for j in range(8):
        sched.append((128, tailb, j))
    for j in range(8):
        sched.append((16, NBLK - 1, j))      # block 31; masked unless owned
    NOPS_ = len(sched)
    assert NOPS_ == NOPS
    woff = np.zeros((128, NOPS), np.int32)
    doff = np.full((128, NOPS), DUMP, np.int32)
    slot_of_op = []
    lbase = {}
    acc = 0
    for b in blocks:
        lbase[b] = acc
        acc += int(ROWOFF[128 * (b + 1)] - ROWOFF[128 * b])
    assert acc == PLOC
    for i, (Cc, b, j) in enumerate(sched):
        slot_of_op.append(blocks.index(b) if b in blocks else 3)
        if b not in blocks:
            continue
        r0 = 128 * b
        for p in range(128):
            r = r0 + p
            L = int(ROWLEN[r])
            if L < Cc:
                continue
            nch = -(-L // Cc)
            if nch > (8 if Cc <= 128 else 2) or j >= nch:
                continue
            base = int(ROWOFF[r] - ROWOFF[r0])
            if j == nch - 1:
                woff[p, i] = r + 1 + L - Cc
                doff[p, i] = base + L - Cc
            else:
                woff[p, i] = r + 1 + Cc * j
                doff[p, i] = base + Cc * j
    # cell op: all cells of rows with L < 16 (block 31 only)
    cellw = np.zeros((128, 1), np.int32)
    celld = np.full((128, 1), DUMP, np.int32)
    cellc = np.zeros((128, 1), np.int32)
    if NBLK - 1 in blocks:
        k = 0
        for r in range(N - 16, N):
            L = int(ROWLEN[r])
            base = int(ROWOFF[r] - ROWOFF[128 * (NBLK - 1)])
            for j in range(L):
                assert k < 128
                cellw[k, 0] = r + 1 + j
                celld[k, 0] = base + j
                cellc[k, 0] = r
                k += 1
    # coverage check per owned-block slot: tiers + cell + fix cover each span
    cov = [np.zeros(PADMAX + max(TIER_CS) + 8, np.int64) for _ in range(4)]
    for i, (Cc, b, j) in enumerate(sched):
        sl = slot_of_op[i]
        for p in range(128):
            if doff[p, i] < DUMP:
                cov[sl][doff[p, i]:doff[p, i] + Cc] += 1
    for k in range(128):
        if celld[k, 0] < DUMP:
            cov[3][celld[k, 0]] += 1
    # fix op covers the last cell of every row; slot j writes block[j] tensor
    srcrow = _triu_src_rows()
    fixw = np.zeros((128, 4), np.int32)
    fixd = np.full((128, 4), DUMP, np.int32)
    for jj, b in enumerate(blocks):
        for p in range(128):
            r = 128 * b + p
            if r < N - 1:
                fixw[p, jj] = srcrow[r]
                fixd[p, jj] = int(ROWOFF[r] - ROWOFF[128 * b]) \
                    + int(ROWLEN[r]) - 1
                cov[jj][fixd[p, jj]] += 1
    for jj, b in enumerate(blocks):
        assert (cov[jj][:SPAN_B[b]] >= 1).all(), f"core {c} slot {jj} uncovered"
    return woff, doff, slot_of_op, blocks, lbase, fixw, fixd, cellw, celld, cellc


def _build(slot_maps, skip_edge=False):
    """Build the SPMD bass program. slot_maps: per-op owned-slot index lists
    (identical across cores by construction)."""
    nc = bacc.Bacc()
    di = lambda n, s: nc.dram_tensor(n, s, I32, kind="ExternalInput")
    df = lambda n, s: nc.dram_tensor(n, s, F32, kind="ExternalInput")
    # values
    x_d = df("x", [ND, N])          # x transposed on host (layout glue)
    sfull_d = nc.dram_tensor("sfull", [N, DL], BF16, kind="ExternalInput")
    degT_d = df("degT", [128, NBLK])        # deg[128b+p] global
    degL_d = df("degL", [1, DL])            # deg of this core's dst slice
    bm_d = df("bm", [1, N])                 # batch_map as f32
    tcol_d = df("tcol", [G, 1])             # timestep as f32
    freqs_d = df("freqs", [G, 64])          # timestep freqs (replicated rows)
    iota8_d = df("iota8", [G, 1])           # 0..7 column
    iota8r_d = df("iota8r", [128, 512])     # tile(0..7, 64) per partition
    cx_d = df("cx", [M, ND + 2])            # [obj_x | obj_pos | ones | 0pad]
    nuc_d = df("nuc", [128, M // 128])
    cen_d = df("cen", [128, M // 128])
    bkb_d = df("bkb", [128, M // 128])
    obt_d = df("obt", [128, M // 128])
    wnode_d = df("Wnode", [ND, H])
    wcond_d = df("Wcond", [H, H])
    wtime_d = df("Wtime", [H, H])
    wconv1_d = df("Wconv1", [H, H])
    wconv2_d = df("Wconv2", [H, H])
    wout_d = df("Wout", [H, ND])
    bnode_d = df("bnode", [1, H])
    bcond_d = df("bcond", [1, H])
    btime_d = df("btime", [1, H])
    bconv1_d = df("bconv1", [128, 1])
    bconv2_d = df("bconv2", [128, 1])
    bout_d = df("bout", [1, ND])
    bedgev_d = df("bedgev", [1, 2])         # [b_edge, 0]
    wpair_d = df("wpair", [H, 2])           # [w_edge[:H] | w_edge[H:]]
    ewoff_d = di("ewoff", [128, NOPS])      # edge gather offsets (per op)
    edoff_d = di("edoff", [128, NOPS])      # edge scatter offsets (per op)
    cellw_d = di("cellw", [128, 1])
    celld_d = di("celld", [128, 1])
    cellc_d = di("cellc", [128, 1])
    slwoff_d = di("slwoff", [128, 4])       # s_l column gather offsets
    fixw_d = di("fixw", [128, 4])           # last-cell fix: s_l source rows
    fixd_d = di("fixd", [128, 4])           # last-cell fix: local dests
    sr1off_d = di("sr1off", [128, 1])       # all N-1 (gather s_r[N-1] col)
    # outputs
    nout_d = nc.dram_tensor("nout", [DL, ND], F32, kind="ExternalOutput")
    elog_ts = [nc.dram_tensor(f"elog{j}", [PADMAX, 1], F32,
                              kind="ExternalOutput") for j in range(4)]
    # internal
    h1loc_d = nc.dram_tensor("h1loc", [H, DL], F32)
    h1all_d = nc.dram_tensor("h1all", [NCORES, H, DL], F32, addr_space="Shared")
    slrloc_d = nc.dram_tensor("slrloc", [2, DL], F32)
    slrall_d = nc.dram_tensor("slrall", [NCORES, 2, DL], F32, addr_space="Shared")
    sl_d = nc.dram_tensor("sl_d", [N, 1], F32)
    sr_d = nc.dram_tensor("sr_d", [N, 1], F32)

    MT = 64  # M // 128 tiles

    with tile.TileContext(nc) as tc:
        with (
            tc.tile_pool(name="cst", bufs=1) as cst,
            tc.tile_pool(name="wts", bufs=1) as wts,
            tc.tile_pool(name="ssb", bufs=1) as ssb,
            tc.tile_pool(name="ps", bufs=2, space="PSUM") as ps,
            tc.tile_pool(name="pst", bufs=2, space="PSUM") as pst,
        ):
            ident = cst.tile([128, 128], F32)
            make_identity(nc, ident[:])
            ident_m = cst.tile([128, 128], MMDT)
            nc.vector.tensor_copy(out=ident_m[:], in_=ident[:])
            ones = cst.tile([128, 520], F32)
            nc.gpsimd.memset(ones[:], 1.0)
            ones_m = cst.tile([128, 8], MMDT)
            nc.vector.tensor_copy(out=ones_m[:], in_=ones[:, 0:8])
            pit = cst.tile([128, 1], F32)
            nc.gpsimd.memset(pit[:], float(np.pi))
            bc = (lambda ap: ap.bitcast(F32R)) if MMDT == F32R else (lambda ap: ap)
            ld = (nc.gpsimd.dma_start if MMDT not in (F32, F32R)
                  else nc.sync.dma_start)

            # ---------------- load S_T into SBUF [128, 32, 512] ----------------
            s_sb = ssb.tile([128, NBLK, DL], MMDT)
            nc.gpsimd.dma_start(
                out=s_sb[:], in_=sfull_d[:].rearrange("(t p) d -> p t d", p=128))

            # ---------------- weights / misc loads ----------------
            ld = (nc.gpsimd.dma_start if MMDT != F32 else nc.sync.dma_start)
            wnode = wts.tile([ND, H], MMDT)
            ld(out=wnode[:], in_=wnode_d[:])
            wcond = wts.tile([H, H], MMDT)
            ld(out=wcond[:], in_=wcond_d[:])
            wtime = wts.tile([H, H], MMDT)
            ld(out=wtime[:], in_=wtime_d[:])
            wconv1 = wts.tile([H, H], MMDT)
            ld(out=wconv1[:], in_=wconv1_d[:])
            wconv2 = wts.tile([H, H], MMDT)
            ld(out=wconv2[:], in_=wconv2_d[:])
            wout = wts.tile([H, ND], F32)
            nc.sync.dma_start(out=wout[:], in_=wout_d[:])
            wpair = wts.tile([H, 2], F32)
            nc.sync.dma_start(out=wpair[:], in_=wpair_d[:])
            bnode = wts.tile([1, H], MMDT)
            ld(out=bnode[:], in_=bnode_d[:])
            bcond = wts.tile([1, H], MMDT)
            ld(out=bcond[:], in_=bcond_d[:])
            btime = wts.tile([1, H], MMDT)
            ld(out=btime[:], in_=btime_d[:])
            bconv1 = wts.tile([128, 1], F32)
            nc.sync.dma_start(out=bconv1[:], in_=bconv1_d[:])
            bconv2 = wts.tile([128, 1], F32)
            nc.sync.dma_start(out=bconv2[:], in_=bconv2_d[:])
            bout = wts.tile([1, ND], F32)
            nc.sync.dma_start(out=bout[:], in_=bout_d[:])
            bedgev = wts.tile([1, 2], F32)
            nc.sync.dma_start(out=bedgev[:], in_=bedgev_d[:])

            # ---------------- x (pre-transposed on host) ----------------
            early_ctx = ExitStack()
            early = early_ctx.enter_context(tc.tile_pool(name="early", bufs=1))
            xt_sb = early.tile([ND, N], MMDT)
            ld(out=xt_sb[:], in_=bc(x_d[:]))

            # ---------------- dinv (global, col layout) ----------------
            degT = wts.tile([128, NBLK], F32)
            nc.sync.dma_start(out=degT[:], in_=degT_d[:])
            dinvT = wts.tile([128, NBLK], F32)
            nc.scalar.activation(dinvT[:], degT[:], AF.Sqrt)
            nc.vector.reciprocal(out=dinvT[:], in_=dinvT[:])
            # dinv for local dst slice, replicated [128, DL]
            degL = wts.tile([1, DL], F32)
            nc.sync.dma_start(out=degL[:], in_=degL_d[:])
            dinvLp = ps.tile([128, DL], F32, space="PSUM", tag="big")
            for u in range(4):
                nc.tensor.matmul(dinvLp[:, 128 * u:128 * (u + 1)],
                                 ones[0:1, 0:128], degL[:, 128 * u:128 * (u + 1)],
                                 start=True, stop=True)
            dinvL = wts.tile([128, DL], F32)
            nc.scalar.activation(dinvL[:], dinvLp[:], AF.Sqrt)
            nc.vector.reciprocal(out=dinvL[:], in_=dinvL[:])

            # ---------------- selmap_T [8, N] ----------------
            bmrow = wts.tile([1, N], F32)
            nc.sync.dma_start(out=bmrow[:], in_=bm_d[:])
            iota8 = wts.tile([G, 1], F32)
            nc.sync.dma_start(out=iota8[:], in_=iota8_d[:])
            selmap = early.tile([G, N], MMDT)
            for q in range(8):
                bmp = pst.tile([G, 512], F32, space="PSUM", tag="tp")
                nc.tensor.matmul(bmp[:], ones[0:1, 0:G],
                                 bmrow[:, 512 * q:512 * (q + 1)],
                                 start=True, stop=True)
                nc.vector.tensor_tensor(
                    out=selmap[:, 512 * q:512 * (q + 1)], in0=bmp[:],
                    in1=iota8[:].to_broadcast([G, 512]), op=ALU.is_equal)

            # ---------------- time embedding ----------------
            tcol = wts.tile([G, 1], F32)
            nc.sync.dma_start(out=tcol[:], in_=tcol_d[:])
            freqs = wts.tile([G, 64], F32)
            nc.sync.dma_start(out=freqs[:], in_=freqs_d[:])
            targ = wts.tile([G, 64], F32)
            nc.vector.tensor_tensor(out=targ[:], in0=freqs[:],
                                    in1=tcol[:].to_broadcast([G, 64]), op=ALU.mult)

            P2 = float(2 * np.pi)

            def reduce_to_pi(xs):
                # range-reduce xs to (-pi, pi] (robust to cast rounding mode)
                y = wts.tile([G, 64], F32, tag="rr_y")
                nc.vector.tensor_scalar(out=y[:], in0=xs[:], scalar1=1.0 / P2,
                                        scalar2=None, op0=ALU.mult)
                yi = wts.tile([G, 64], I32, tag="rr_yi")
                nc.vector.tensor_copy(out=yi[:], in_=y[:])
                yf = wts.tile([G, 64], F32, tag="rr_yf")
                nc.vector.tensor_copy(out=yf[:], in_=yi[:])
                nc.vector.tensor_scalar(out=yf[:], in0=yf[:], scalar1=-P2,
                                        scalar2=None, op0=ALU.mult)
                nc.vector.tensor_tensor(out=xs[:], in0=xs[:], in1=yf[:],
                                        op=ALU.add)
                adj = wts.tile([G, 64], F32, tag="rr_adj")
                nc.vector.tensor_scalar(out=adj[:], in0=xs[:],
                                        scalar1=float(np.pi), scalar2=None,
                                        op0=ALU.is_gt)
                nc.vector.tensor_scalar(out=adj[:], in0=adj[:], scalar1=-P2,
                                        scalar2=None, op0=ALU.mult)
                nc.vector.tensor_tensor(out=xs[:], in0=xs[:], in1=adj[:],
                                        op=ALU.add)
                nc.vector.tensor_scalar(out=adj[:], in0=xs[:],
                                        scalar1=float(-np.pi), scalar2=None,
                                        op0=ALU.is_lt)
                nc.vector.tensor_scalar(out=adj[:], in0=adj[:], scalar1=P2,
                                        scalar2=None, op0=ALU.mult)
                nc.vector.tensor_tensor(out=xs[:], in0=xs[:], in1=adj[:],
                                        op=ALU.add)

            temb = wts.tile([G, H], F32)
            # cos(x) = sin(x + pi/2) -> [:, :64]
            cosm = wts.tile([G, 64], F32)
            nc.vector.tensor_scalar(out=cosm[:], in0=targ[:],
                                    scalar1=float(np.pi / 2), scalar2=None,
                                    op0=ALU.add)
            reduce_to_pi(cosm)
            nc.scalar.activation(temb[:, 0:64], cosm[:], AF.Sin)
            # sin -> [:, 64:]
            sinm = wts.tile([G, 64], F32)
            nc.vector.tensor_copy(out=sinm[:], in_=targ[:])
            reduce_to_pi(sinm)
            nc.scalar.activation(temb[:, 64:128], sinm[:], AF.Sin)
            tembT_p = pst.tile([H, G], F32, space="PSUM", tag="tp")
            nc.tensor.transpose(out=tembT_p[:], in_=temb[:], identity=ident[0:G, 0:G])
            tembT = wts.tile([H, G], MMDT)
            nc.vector.tensor_copy(out=tembT[:], in_=tembT_p[:])

            # ---------------- cond pooling ----------------
            cx = early.tile([128, MT, ND + 2], MMDT)
            ld(out=cx[:], in_=bc(cx_d[:].rearrange("(t p) d -> p t d", p=128)))
            nuc = wts.tile([128, MT], F32)
            nc.sync.dma_start(out=nuc[:], in_=nuc_d[:])
            cen = wts.tile([128, MT], F32)
            nc.sync.dma_start(out=cen[:], in_=cen_d[:])
            bkb = wts.tile([128, MT], F32)
            nc.sync.dma_start(out=bkb[:], in_=bkb_d[:])
            obt = wts.tile([128, MT], F32)
            nc.sync.dma_start(out=obt[:], in_=obt_d[:])
            iota8r = wts.tile([128, 512], F32)
            nc.sync.dma_start(out=iota8r[:], in_=iota8r_d[:])
            isl = wts.tile([128, MT], F32)
            nc.vector.tensor_scalar(out=isl[:], in0=nuc[:], scalar1=0.0,
                                    scalar2=None, op0=ALU.is_equal)
            isr = wts.tile([128, MT], F32)
            nc.vector.tensor_scalar(out=isr[:], in0=nuc[:], scalar1=2.0,
                                    scalar2=None, op0=ALU.is_equal)
            isb = wts.tile([128, MT], F32)
            nc.vector.tensor_scalar(out=isb[:], in0=bkb[:], scalar1=0.0,
                                    scalar2=None, op0=ALU.is_equal)
            t1 = wts.tile([128, MT], F32)
            nc.vector.tensor_tensor(out=t1[:], in0=cen[:], in1=isb[:], op=ALU.mult)
            t2 = wts.tile([128, MT], F32)
            nc.vector.tensor_tensor(out=t2[:], in0=isr[:], in1=isb[:], op=ALU.mult)
            cm = wts.tile([128, MT], F32)
            nc.vector.tensor_tensor(out=cm[:], in0=isl[:], in1=t1[:], op=ALU.max)
            nc.vector.tensor_tensor(out=cm[:], in0=cm[:], in1=t2[:], op=ALU.max)
            sel = early.tile([128, MT, G], F32)
            nc.vector.tensor_tensor(
                out=sel[:], in0=obt[:, :, None].to_broadcast([128, MT, G]),
                in1=iota8r[:].rearrange("p (t g) -> p t g", g=G), op=ALU.is_equal)
            nc.vector.tensor_tensor(
                out=sel[:], in0=sel[:],
                in1=cm[:, :, None].to_broadcast([128, MT, G]), op=ALU.mult)
            sel_m = early.tile([128, MT, G], MMDT)
            nc.vector.tensor_copy(out=sel_m[:], in_=sel[:])
            poolp = ps.tile([G, ND + 2], F32, space="PSUM", tag="acc8")
            for t in range(MT):
                nc.tensor.matmul(poolp[:], sel_m[:, t, :], cx[:, t, :],
                                 start=(t == 0), stop=(t == MT - 1))
            pooled = wts.tile([G, ND + 2], F32)
            nc.vector.tensor_copy(out=pooled[:], in_=poolp[:])
            cnt = wts.tile([G, 1], F32)
            nc.vector.tensor_scalar(out=cnt[:], in0=pooled[:, ND:ND + 1],
                                    scalar1=1.0, scalar2=None, op0=ALU.max)
            rcnt = wts.tile([G, 1], F32)
            nc.vector.reciprocal(out=rcnt[:], in_=cnt[:])
            pxT_p = pst.tile([ND, G], F32, space="PSUM", tag="tp")
            nc.tensor.transpose(out=pxT_p[:], in_=pooled[:, 0:ND],
                                identity=ident[0:G, 0:G])
            pxT = wts.tile([ND, G], MMDT)
            nc.vector.tensor_copy(out=pxT[:], in_=pxT_p[:])
            cT_p = pst.tile([1, G], F32, space="PSUM", tag="tp")
            nc.tensor.transpose(out=cT_p[:], in_=pooled[:, ND:ND + 1],
                                identity=ident[0:G, 0:G])
            cT = wts.tile([1, G], MMDT)
            nc.vector.tensor_copy(out=cT[:], in_=cT_p[:])
            p1 = ps.tile([G, H], F32, space="PSUM", tag="acc8")
            nc.tensor.matmul(p1[:], pxT[:], wnode[:], start=True, stop=False)
            nc.tensor.matmul(p1[:], cT[:], bnode[:], start=False, stop=True)
            pmean = wts.tile([G, H], F32)
            nc.vector.tensor_tensor(out=pmean[:], in0=p1[:],
                                    in1=rcnt[:].to_broadcast([G, H]), op=ALU.mult)
            pmT_p = pst.tile([H, G], F32, space="PSUM", tag="tp")
            nc.tensor.transpose(out=pmT_p[:], in_=pmean[:], identity=ident[0:G, 0:G])
            pmT = wts.tile([H, G], MMDT)
            nc.vector.tensor_copy(out=pmT[:], in_=pmT_p[:])

            # ---------------- combined [8, H] ----------------
            combp = ps.tile([G, H], F32, space="PSUM", tag="acc8")
            nc.tensor.matmul(combp[:], tembT[:], wtime[:], start=True, stop=False)
            nc.tensor.matmul(combp[:], pmT[:], wcond[:], start=False, stop=False)
            nc.tensor.matmul(combp[:], ones_m[0:1, 0:G], btime[:],
                             start=False, stop=False)
            nc.tensor.matmul(combp[:], ones_m[0:1, 0:G], bcond[:],
                             start=False, stop=False)
            nc.tensor.matmul(combp[:], ones_m[0:1, 0:G], bnode[:],
                             start=False, stop=True)
            comb = wts.tile([G, H], MMDT)
            nc.vector.tensor_copy(out=comb[:], in_=combp[:])

            # ---------------- node_emb_T then layers ----------------
            neT = ssb.tile([H, N], MMDT)
            for q in range(8):
                nep = ps.tile([H, 512], F32, space="PSUM", tag="big")
                nc.tensor.matmul(nep[:], wnode[:],
                                 xt_sb[:, 512 * q:512 * (q + 1)],
                                 start=True, stop=False)
                nc.tensor.matmul(nep[:], comb[:],
                                 selmap[:, 512 * q:512 * (q + 1)],
                                 start=False, stop=True)
                nc.vector.tensor_copy(out=neT[:, 512 * q:512 * (q + 1)], in_=nep[:])
            early_ctx.close()

            h1 = ssb.tile([H, DL], F32)
            h2 = ssb.tile([H, DL], F32)
            h1T = ssb.tile([H, N], MMDT)
            ztiles = ssb.tile([128, NBLK, H], MMDT, tag="ztiles")

            for layer in range(2):
                wconv = wconv1 if layer == 0 else wconv2
                bconv = bconv1 if layer == 0 else bconv2
                src = neT if layer == 0 else h1T
                hout = h1 if layer == 0 else h2
                # proj_T = Wconv^T @ src, written back in place chunk by chunk
                for q in range(8):
                    prp = ps.tile([H, 512], F32, space="PSUM", tag="big")
                    nc.tensor.matmul(prp[:], wconv[:],
                                     src[:, 512 * q:512 * (q + 1)],
                                     start=True, stop=True)
                    nc.vector.tensor_copy(out=src[:, 512 * q:512 * (q + 1)],
                                          in_=prp[:])
                # z tiles = transpose(proj_T) * dinv col
                for t in range(NBLK):
                    ztp = pst.tile([128, 128], MMDT, space="PSUM", tag="ztp")
                    nc.tensor.transpose(out=ztp[:],
                                        in_=src[:, 128 * t:128 * (t + 1)],
                                        identity=ident_m[:])
                    nc.vector.tensor_tensor(
                        out=ztiles[:, t, :], in0=ztp[:],
                        in1=dinvT[:, t:t + 1].to_broadcast([128, H]), op=ALU.mult)
                # aggregate: agg_T[h, d_local] += z_t^T S_t
                aggp = ps.tile([H, DL], F32, space="PSUM", tag="big")
                for t in range(NBLK):
                    nc.tensor.matmul(aggp[:], ztiles[:, t, :], s_sb[:, t, :],
                                     start=(t == 0), stop=(t == NBLK - 1))
                # h = relu(agg * dinvL + b)
                nc.vector.tensor_tensor(out=hout[:], in0=aggp[:], in1=dinvL[:],
                                        op=ALU.mult)
                nc.scalar.activation(hout[:], hout[:], AF.Relu, bias=bconv[:],
                                     scale=1.0)
                if layer == 0:
                    nc.sync.dma_start(out=h1loc_d[:], in_=hout[:])
                    nc.gpsimd.collective_compute(
                        "AllGather", ALU.bypass,
                        replica_groups=[list(range(NCORES))],
                        ins=[h1loc_d[:]], outs=[h1all_d[:]])
                    for q in range(NCORES):
                        ld(out=h1T[:, DL * q:DL * (q + 1)],
                           in_=bc(h1all_d[q]))

            # ---------------- outputs: node noise ----------------
            no_sb = wts.tile([128, 4, ND], F32)
            for u in range(4):
                nop = pst.tile([128, ND], F32, space="PSUM", tag="ztp")
                nc.tensor.matmul(nop[:], h2[:, 128 * u:128 * (u + 1)],
                                 wout[:], start=True, stop=False)
                nc.tensor.matmul(nop[:], ones[0:1, 0:128], bout[:],
                                 start=False, stop=True)
                nc.vector.tensor_copy(out=no_sb[:, u, :], in_=nop[:])
            nc.sync.dma_start(
                out=nout_d[:].rearrange("(u p) d -> p u d", p=128), in_=no_sb[:])

            # ---------------- s_l / s_r + allgather ----------------
            slrp = ps.tile([2, DL], F32, space="PSUM", tag="big")
            nc.tensor.matmul(slrp[:], wpair[:], h2[:],
                             start=True, stop=False)
            nc.tensor.matmul(slrp[:], bedgev[:], ones[0:1, 0:DL],
                             start=False, stop=True)
            slr = wts.tile([2, DL], F32)
            nc.vector.tensor_copy(out=slr[:], in_=slrp[:])
            nc.sync.dma_start(out=slrloc_d[:], in_=slr[:])
            nc.gpsimd.collective_compute(
                "AllGather", ALU.bypass, replica_groups=[list(range(NCORES))],
                ins=[slrloc_d[:]], outs=[slrall_d[:]])
            nc.sync.dma_start(
                out=sl_d[:, 0].rearrange("(a b) -> a b", a=NCORES),
                in_=slrall_d[:, 0, :])
            nc.sync.dma_start(
                out=sr_d[:, 0].rearrange("(a b) -> a b", a=NCORES),
                in_=slrall_d[:, 1, :])

        # ---------------- edge logits ----------------
        if skip_edge:
            return nc
        with (
            tc.tile_pool(name="esb", bufs=4) as esb,
            tc.tile_pool(name="ecst", bufs=1) as ecst,
        ):
            ewoff = ecst.tile([128, NOPS], I32)
            nc.sync.dma_start(out=ewoff[:], in_=ewoff_d[:])
            edoff = ecst.tile([128, NOPS], I32)
            nc.sync.dma_start(out=edoff[:], in_=edoff_d[:])
            slwoff = ecst.tile([128, 4], I32)
            nc.sync.dma_start(out=slwoff[:], in_=slwoff_d[:])
            slcols = ecst.tile([128, 4], F32)
            for j in range(4):
                nc.gpsimd.indirect_dma_start(
                    out=slcols[:, j:j + 1], out_offset=None, in_=sl_d[:],
                    in_offset=IndirectOffsetOnAxis(ap=slwoff[:, j:j + 1],
                                                   axis=0))
            for i, Cc in enumerate(TIER_CS):
                slot = slot_maps[i]
                pay = esb.tile([128, 2048], F32, tag="pay")
                nc.gpsimd.indirect_dma_start(
                    out=pay[:, 0:Cc], out_offset=None, in_=sr_d[:],
                    in_offset=IndirectOffsetOnAxis(ap=ewoff[:, i:i + 1],
                                                   axis=0))
                nc.vector.tensor_tensor(
                    out=pay[:, 0:Cc], in0=pay[:, 0:Cc],
                    in1=slcols[:, slot:slot + 1].to_broadcast([128, Cc]),
                    op=ALU.add)
                nc.gpsimd.indirect_dma_start(
                    out=elog_ts[slot][:],
                    out_offset=IndirectOffsetOnAxis(ap=edoff[:, i:i + 1],
                                                    axis=0),
                    in_=pay[:, 0:Cc], in_offset=None)
            # cell op (rows with L < 16; only the core owning block 31)
            cellw = ecst.tile([128, 1], I32)
            nc.sync.dma_start(out=cellw[:], in_=cellw_d[:])
            celld = ecst.tile([128, 1], I32)
            nc.sync.dma_start(out=celld[:], in_=celld_d[:])
            cellc = ecst.tile([128, 1], I32)
            nc.sync.dma_start(out=cellc[:], in_=cellc_d[:])
            cw = ecst.tile([128, 1], F32)
            nc.gpsimd.indirect_dma_start(
                out=cw[:], out_offset=None, in_=sr_d[:],
                in_offset=IndirectOffsetOnAxis(ap=cellw[:], axis=0))
            cc = ecst.tile([128, 1], F32)
            nc.gpsimd.indirect_dma_start(
                out=cc[:], out_offset=None, in_=sl_d[:],
                in_offset=IndirectOffsetOnAxis(ap=cellc[:], axis=0))
            cv = ecst.tile([128, 1], F32)
            nc.vector.tensor_tensor(out=cv[:], in0=cw[:], in1=cc[:], op=ALU.add)
            nc.gpsimd.indirect_dma_start(
                out=elog_ts[3][:],
                out_offset=IndirectOffsetOnAxis(ap=celld[:], axis=0),
                in_=cv[:], in_offset=None)
            # last-cell fix (ordered after tier scatters via DRAM WAW deps)
            fixw = ecst.tile([128, 4], I32)
            nc.sync.dma_start(out=fixw[:], in_=fixw_d[:])
            fixd = ecst.tile([128, 4], I32)
            nc.sync.dma_start(out=fixd[:], in_=fixd_d[:])
            sr1off = ecst.tile([128, 1], I32)
            nc.sync.dma_start(out=sr1off[:], in_=sr1off_d[:])
            g2 = ecst.tile([128, 1], F32)
            nc.gpsimd.indirect_dma_start(
                out=g2[:], out_offset=None, in_=sr_d[:],
                in_offset=IndirectOffsetOnAxis(ap=sr1off[:], axis=0))
            for j in range(4):
                g1 = ecst.tile([128, 1], F32, tag="g1")
                nc.gpsimd.indirect_dma_start(
                    out=g1[:], out_offset=None, in_=sl_d[:],
                    in_offset=IndirectOffsetOnAxis(ap=fixw[:, j:j + 1],
                                                   axis=0))
                fv = ecst.tile([128, 1], F32, tag="fv")
                nc.vector.tensor_tensor(out=fv[:], in0=g1[:], in1=g2[:],
                                        op=ALU.add)
                nc.gpsimd.indirect_dma_start(
                    out=elog_ts[j][:],
                    out_offset=IndirectOffsetOnAxis(ap=fixd[:, j:j + 1],
                                                    axis=0),
                    in_=fv[:], in_offset=None)
    return nc


_CACHE = {}
_TRIU = {}


def _triu_src_rows():
    """Per-row source row for the LAST cell of each triu row, exactly as
    jnp.triu_indices produces it (it wraps the last cell of row r to
    (r+1, -1) for some rows). Input-independent; computed once via jax."""
    if "src" in _TRIU:
        return _TRIU["src"]
    import jax
    import jax.numpy as jnp
    cpu = jax.local_devices(backend="cpu")[0]
    with jax.default_device(cpu):
        jrow, jcol = jnp.triu_indices(N, k=1)
        jrow = np.asarray(jrow).astype(np.int64)
        jcol = np.asarray(jcol).astype(np.int64)
    # verify: all non-last positions are plain row-major
    rr = np.searchsorted(ROWOFF, np.arange(jrow.size), side="right") - 1
    cc = np.arange(jrow.size) - ROWOFF[rr] + rr + 1
    lastpos = (ROWOFF[: N - 1] + ROWLEN[: N - 1] - 1).astype(np.int64)
    ismid = np.ones(jrow.size, bool)
    ismid[lastpos] = False
    assert (jrow[ismid] == rr[ismid]).all() and (jcol[ismid] == cc[ismid]).all()
    assert np.isin(jcol[lastpos] % N, [N - 1]).all()
    _TRIU["src"] = jrow[lastpos]          # s_l row index for last cell of row r
    return _TRIU["src"]


def _prep_host(inputs):
    """Split/shard inputs into 8 per-core in_maps (host: integer index prep,
    dtype casts, reshapes only)."""
    src = np.asarray(inputs["edge_index"][0], np.int64)
    dst = np.asarray(inputs["edge_index"][1], np.int64)
    loops = np.arange(N, dtype=np.int64)
    srcA = np.concatenate([src, loops])
    dstA = np.concatenate([dst, loops])
    deg = np.bincount(dstA, minlength=N).astype(np.float32)

    x = np.ascontiguousarray(np.asarray(inputs["x"], np.float32).T)
    bm = np.asarray(inputs["batch_map"], np.float32)[None, :]
    tcol = np.asarray(inputs["timestep"], np.float32)[:, None]
    half = H // 2
    freqs = np.exp(-np.log(MAX_PERIOD) *
                   np.arange(half, dtype=np.float32) / half)
    freqs_rep = np.tile(freqs[None, :], (G, 1)).astype(np.float32)
    iota8 = np.arange(G, dtype=np.float32)[:, None]
    iota8r = np.tile(np.arange(G, dtype=np.float32)[None, :], (128, 64))
    cx = np.concatenate([
        np.asarray(inputs["obj_x"], np.float32),
        np.asarray(inputs["obj_pos"], np.float32),
        np.ones((M, 1), np.float32),
        np.zeros((M, 1), np.float32)], axis=1)
    colsM = lambda v: np.ascontiguousarray(
        np.asarray(v, np.float32).reshape(M // 128, 128).T)
    degT = np.ascontiguousarray(deg.reshape(NBLK, 128).T)
    w_edge = np.asarray(inputs["w_edge"], np.float32)
    wpair = np.stack([w_edge[:H], w_edge[H:]], axis=1)
    bedgev = np.array([[float(np.asarray(inputs["b_edge"])), 0.0]], np.float32)

    base = dict(
        x=x, bm=bm, tcol=tcol, freqs=freqs_rep, iota8=iota8, iota8r=iota8r,
        cx=cx,
        nuc=colsM(inputs["obj_nucleotide_mask"]),
        cen=colsM(inputs["obj_central_mask"]),
        bkb=colsM(inputs["obj_backbone_mask"]),
        obt=colsM(inputs["obj_batch"]),
        degT=degT,

        Wnode=np.asarray(inputs["W_node"], np.float32),
        Wcond=np.asarray(inputs["W_cond"], np.float32),
        Wtime=np.asarray(inputs["W_time"], np.float32),
        Wconv1=np.asarray(inputs["W_conv1"], np.float32),
        Wconv2=np.asarray(inputs["W_conv2"], np.float32),
        Wout=np.asarray(inputs["W_out"], np.float32),
        bnode=np.asarray(inputs["b_node"], np.float32)[None, :],
        bcond=np.asarray(inputs["b_cond"], np.float32)[None, :],
        btime=np.asarray(inputs["b_time"], np.float32)[None, :],
        bconv1=np.asarray(inputs["b_conv1"], np.float32)[:, None],
        bconv2=np.asarray(inputs["b_conv2"], np.float32)[:, None],
        bout=np.asarray(inputs["b_out"], np.float32)[None, :],
        bedgev=bedgev, wpair=wpair,
    )

    in_maps = []
    slot_maps = None
    for c in range(NCORES):
        m = dict(base)
        lo = DL * c
        sel = (dstA >= lo) & (dstA < lo + DL)
        import ml_dtypes
        sfull = np.zeros(N * DL, np.float32)
        np.add.at(sfull, srcA[sel] * DL + (dstA[sel] - lo), 1.0)
        m["sfull"] = sfull.reshape(N, DL).astype(ml_dtypes.bfloat16)
        m["degL"] = deg[lo:lo + DL][None, :]
        (woff, doff, slots, blocks, lbase, fixw, fixd,
         cellw, celld, cellc) = _edge_tables(c)
        m["ewoff"] = woff
        m["edoff"] = doff
        m["fixw"] = fixw
        m["fixd"] = fixd
        m["cellw"] = cellw
        m["celld"] = celld
        m["cellc"] = cellc
        m["sr1off"] = np.full((128, 1), N - 1, np.int32)
        slw = np.zeros((128, 4), np.int32)
        for j, b in enumerate(blocks):
            slw[:, j] = 128 * b + np.arange(128)
        m["slwoff"] = slw
        if slot_maps is None:
            slot_maps = slots
        else:
            assert slot_maps == slots
        in_maps.append(m)
    return in_maps, slot_maps


def _assemble(results):
    nout = np.concatenate([results[c]["nout"] for c in range(NCORES)], axis=0)
    elog = np.empty(P_TOTAL, np.float32)
    for c in range(NCORES):
        for j, b in enumerate(OWNED[c]):
            span = SPAN_B[b]
            elog[int(ROWOFF[128 * b]):int(ROWOFF[128 * (b + 1)])] = \
                results[c][f"elog{j}"].ravel()[:span]
    return nout, elog


def kernel(**inputs):
    from concourse.bass_utils import run_bass_kernel_spmd
    in_maps, slot_maps = _prep_host(inputs)
    key = "prog"
    if key not in _CACHE:
        nc = _build(slot_maps)
        nc.finalize()
        _CACHE[key] = nc
    nc = _CACHE[key]
    res = run_bass_kernel_spmd(nc, in_maps, list(range(NCORES)))
    return _assemble(res.results)
